# revision 19
# baseline (speedup 1.0000x reference)
"""BitLinear fake-quant GEMM on 8 TRN2 NeuronCores — fp8 DoubleRow edition.

Reference math:
  abs_mean  = mean(|W|);  thr = 0.7*abs_mean
  Wq        = sign(W) * (|W| >= thr)            (ternary)
  scale_w   = abs_mean / (mean(Wq != 0) + 1e-8)
  sx        = 127 / max(|X|)
  Xq        = round(X * sx)                      (integer valued, |.| <= 127)
  out       = (Xq @ Wq^T) * scale_w / sx

Sharding: data-parallel over tokens (8192/8 = 1024 columns of X^T per core);
W is replicated.  The host hands each core a PRE-TRANSPOSED bf16 x shard and
fp32 w^T, so both matmul operands have the contraction dim on partitions.

The GEMM runs on the PE in fp8e4 (e4m3) DoubleRow mode: one matmul
instruction contracts TWO 128-deep k-chunks at 0.5 cycles/row — 2x the bf16
FLOP rate.  Exactness: Xq (ints, |.|<=127) is split Xq = A + B with
A = fp8_rne(Xq) (e4m3-exact) and B = Xq - A (integer in [-4,4], fp8-exact);
Wq in {-1,0,1} is fp8-exact.  A@Wq + B@Wq accumulated in fp32 PSUM
reproduces Xq@Wq exactly (all products integers, sums < 2^24).  Net PE time
halves vs a bf16 kernel.

x is pre-converted to bf16 on the host (pure dtype cast, no stats): halves
x DMA and SBUF.  |x|-max is computed on device from the bf16 values; sx then
differs from the fp32-max reference by <= 2^-9 relative, worth ~3.6 max
output deviation against the 11.2 allowed by rel_err 2e-2.

Schedule (single 360 B/ns DMA device; every engine queue is in-order, and a
DMA whose semaphore wait is pending BLOCKS its queue's dispatch — used
deliberately below):
  sync-q: wsl x4, x pass-1 x16, collective-input hop (blocks until local
          stats land, keeping later transfers from queueing ahead of it at
          the FIFO DMA device), p0-q0 + two re-reads, collective-output hop
          (blocks until the AllGather lands), then p0-q1..q3 interleaved
          with re-reads, panels 1-7 quarters.  scalar-q: bf16 output writes.
  DVE:  x |max| pass (tensor_scalar + max accum_out at the 2x rate, |x| to a
        victim tile so the signed tiles survive), u = x*sx + MAGIC (2x),
        B = (u-MAGIC) - A, ternary wq for panels 1-7 (panel 1's interleaved
        into the last pairs).
  ACT:  |wsl| abs + sum accum_out, A = fp8(u - MAGIC), PSUM->bf16 copies.
  Pool: partition reduces/broadcasts, collective dispatch, b2 = (w<=-thr)
        for all panels, and panel-0's wq (keeps DVE free for the pairs the
        moment sx lands).
  PE:   panel 0 pair-major in production order (each arriving (A,B) pair
        feeds all 8 PSUM banks), panels 1-7 quarter-major.
  The last 3 pairs' x tiles are still resident from pass 1, so they need no
  re-read and quantize first, at sx+0.

Stats: per-core |W|-slice sum + |x|-shard max, one 2-scalar AllGather, local
reduce.  nnz (host-consumed only, for scale_w) is counted on the host by
replaying the device's exact fp32 threshold compare.  Output is written
bf16, tile-chunked; the host upcasts, scales by scale_w/sx and permutes.
"""

from contextlib import ExitStack

import numpy as np
import ml_dtypes

import concourse.bass as bass
import concourse.bass_isa as bass_isa
import concourse.tile as tile
from concourse import bacc, mybir
from concourse.bass import ts as _ts
from concourse.bass_utils import run_bass_kernel_spmd

P = 128
T, I, O = 8192, 4096, 4096  # tokens, in_features, out_features
NC = 8
TSH = T // NC  # 1024 token columns per core
ISL = I // NC  # 512 wT rows per core for stats
NMM = 512  # matmul moving free dim (one fp32 PSUM bank)
GF = 4096  # W staging tile free size (one [128, 4096] fp32 tile = 2 MB)
NPAIR = 16  # k-chunk pairs (32 chunks of 128 over I=4096)
NRES = 6  # trailing pairs whose pass-1 tiles stay resident (= xin bufs)
MAGIC = 12582912.0  # 1.5 * 2**23: fp32 round-to-nearest-even bias trick

F32 = mybir.dt.float32
BF16 = mybir.dt.bfloat16
FP8 = mybir.dt.float8e4
ALU = mybir.AluOpType
AXX = mybir.AxisListType
ACTF = mybir.ActivationFunctionType
DR = mybir.MatmulPerfMode.DoubleRow

# pass-1 load order: pairs 0..NRES-1 last, so they are the tiles still
# resident in the xin pool when sx lands — and they need quarter 0, which is
# also the first ternary quarter produced
PASS1_ORDER = list(range(NRES, NPAIR)) + list(range(NRES))
PAIR_ORDER = list(range(NPAIR))


def _bitlinear(tc, out, sout, xT, wT, wsl):
    nc = tc.nc
    with ExitStack() as ctx:
        const = ctx.enter_context(tc.tile_pool(name="const", bufs=1))
        statp = ctx.enter_context(tc.tile_pool(name="statp", bufs=1))
        dram = ctx.enter_context(tc.tile_pool(name="dram", bufs=1, space="DRAM"))
        stg = ctx.enter_context(tc.tile_pool(name="stg", bufs=4))    # f32 [128,4096]
        xin = ctx.enter_context(tc.tile_pool(name="xin", bufs=NRES))  # bf16 [128,2048]
        up = ctx.enter_context(tc.tile_pool(name="up", bufs=2))      # f32 [128,2048]
        abp = ctx.enter_context(tc.tile_pool(name="abp", bufs=1))    # fp8 [128,2048] x32
        wqp = ctx.enter_context(tc.tile_pool(name="wqp", bufs=5))    # fp8 [128,4096]
        b2p = ctx.enter_context(tc.tile_pool(name="b2p", bufs=2))    # fp8 [128,4096]
        psum = ctx.enter_context(tc.tile_pool(name="psum", bufs=1, space="PSUM"))
        osb = ctx.enter_context(tc.tile_pool(name="osb", bufs=4))    # bf16 [128,512]

        # Pool-engine consts first so they don't queue behind later Pool work
        nmagic128 = const.tile([P, 1], F32)
        nc.gpsimd.memset(nmagic128[:], -MAGIC)

        def xpair_src(j):
            # xT rows [2j*128, (2j+2)*128) as [128, 2 chunks, 1024 tokens]
            return xT[2 * j * P : (2 * j + 2) * P, :].rearrange(
                "(c p) t -> p c t", p=P
            )

        # ---- Phase 1: |W| slice sum (ACT Abs + accumulator) interleaved
        # with the x |max| pass (DVE tensor_reduce).  The wsl transfers are
        # spread through the x stream so the 2.2us-per-tile reduces pipeline
        # under the DMA stream instead of extending past it ----
        wsum_part = statp.tile([P, 4], F32)
        xmax_part = statp.tile([P, NPAIR], F32)
        x_tiles = [None] * NPAIR

        def emit_wsl(c):
            wt = stg.tile([P, GF], F32, tag="stg", name=f"wsl{c}")
            nc.sync.dma_start(wt[:], wsl[_ts(c, P), :])
            # in-place |w|; accum_out gives the per-partition row sum free
            nc.scalar.activation(
                wt[:], wt[:], ACTF.Abs, accum_out=wsum_part[:, c : c + 1]
            )

        def emit_x1(j):
            xt = xin.tile([P, 2 * TSH], BF16, tag="xin", name=f"x1_{j}")
            nc.sync.dma_start(
                xt[:].rearrange("p (c t) -> p c t", c=2), xpair_src(j)
            )
            nc.vector.tensor_reduce(
                xmax_part[:, j : j + 1], xt[:], axis=AXX.X, op=ALU.max,
                apply_absolute_value=True,
            )
            x_tiles[j] = xt

        for n, j in enumerate(PASS1_ORDER):
            if n in (2, 6, 10, 14):
                emit_wsl((n - 2) // 4)
            emit_x1(j)

        wsum_c = statp.tile([P, 1], F32)
        nc.vector.tensor_reduce(wsum_c[:], wsum_part[:], axis=AXX.X, op=ALU.add)
        xmax_c = statp.tile([P, 1], F32)
        nc.vector.tensor_reduce(xmax_c[:], xmax_part[:], axis=AXX.X, op=ALU.max)
        wsum_a = statp.tile([P, 1], F32)
        nc.gpsimd.partition_all_reduce(
            wsum_a[:], wsum_c[:], channels=P, reduce_op=bass_isa.ReduceOp.add
        )
        xmax_a = statp.tile([P, 1], F32)
        nc.gpsimd.partition_all_reduce(
            xmax_a[:], xmax_c[:], channels=P, reduce_op=bass_isa.ReduceOp.max
        )

        # ---- one tiny AllGather of [wsum, xmax]; reduce locally.  Both DRAM
        # hops ride the sync queue: a waiting DMA blocks the queue's
        # dispatch, so later bulk transfers cannot queue ahead of the hop at
        # the FIFO DMA device — each hop fires the moment its data is ready --
        loc = statp.tile([1, 2], F32)
        nc.vector.tensor_copy(loc[0:1, 0:1], wsum_a[0:1, 0:1])
        nc.vector.tensor_copy(loc[0:1, 1:2], xmax_a[0:1, 0:1])
        cin = dram.tile([1, 2], F32)
        cout = dram.tile([1, 2 * NC], F32)
        nc.sync.dma_start(cin[:], loc[:])
        nc.gpsimd.collective_compute(
            "AllGather", ALU.bypass, replica_groups=[list(range(NC))],
            ins=[cin.opt()], outs=[cout.opt()],
        )

        def emit_wload(op_, q, nm):
            wt = stg.tile([P, GF], F32, tag="stg", name=nm)
            src = wT[
                q * 1024 : (q + 1) * 1024, _ts(op_, NMM)
            ].rearrange("(c p) j -> p c j", p=P)
            nc.sync.dma_start(wt[:].rearrange("p (c j) -> p c j", c=8), src)
            return wt

        def emit_re(j):
            xt = xin.tile([P, 2 * TSH], BF16, tag="xin", name=f"xre{j}")
            nc.sync.dma_start(
                xt[:].rearrange("p (c t) -> p c t", c=2), xpair_src(j)
            )
            x_tiles[j] = xt

        # p0-q0 + two re-reads slot in ahead of the collective's output hop
        p0_wt = [None] * 4
        p0_wt[0] = emit_wload(0, 0, "p0w0")
        emit_re(6)
        emit_re(7)

        gg = statp.tile([1, 2 * NC], F32)
        nc.sync.dma_start(gg[:], cout[:])

        p0_wt[1] = emit_wload(0, 1, "p0w1")
        emit_re(8)
        emit_re(9)
        p0_wt[2] = emit_wload(0, 2, "p0w2")
        emit_re(10)
        emit_re(11)
        p0_wt[3] = emit_wload(0, 3, "p0w3")
        for j in range(12, NPAIR):
            emit_re(j)

        gg3 = gg[:].rearrange("a (r k) -> a r k", k=2)
        gsum = statp.tile([1, 1], F32)
        nc.vector.tensor_reduce(gsum[:], gg3[:, :, 0:1], axis=AXX.XY, op=ALU.add)
        gmax = statp.tile([1, 1], F32)
        nc.vector.tensor_reduce(gmax[:], gg3[:, :, 1:2], axis=AXX.XY, op=ALU.max)

        thr1 = statp.tile([1, 1], F32)
        nc.vector.tensor_scalar(thr1[:], gsum[:], 0.7 / float(O * I), None, op0=ALU.mult)
        nthr1 = statp.tile([1, 1], F32)
        nc.vector.tensor_scalar(nthr1[:], thr1[:], -1.0, None, op0=ALU.mult)
        thr128 = const.tile([P, 1], F32)
        nc.gpsimd.partition_broadcast(thr128[:], thr1[:])
        nthr128 = const.tile([P, 1], F32)
        nc.gpsimd.partition_broadcast(nthr128[:], nthr1[:])

        gmax_c = statp.tile([1, 1], F32)
        nc.vector.tensor_scalar(gmax_c[:], gmax[:], 1e-12, None, op0=ALU.max)
        rec1 = statp.tile([1, 1], F32)
        nc.vector.reciprocal(rec1[:], gmax_c[:])
        sx1 = statp.tile([1, 1], F32)
        nc.vector.tensor_scalar(sx1[:], rec1[:], 127.0, None, op0=ALU.mult)
        sx128 = const.tile([P, 1], F32)
        nc.gpsimd.partition_broadcast(sx128[:], sx1[:])

        def emit_wq_dve(wt, b2, nm):
            wq = wqp.tile([P, GF], FP8, tag="wq", name=nm)
            nc.vector.scalar_tensor_tensor(
                wq[:], wt[:], thr128[:], b2[:], op0=ALU.is_ge, op1=ALU.subtract
            )
            return wq

        # ---- panel-0 quant: quarters 0+1 ternary on DVE ahead of the pair
        # stream (q0's b2 too — no Pool dependency on the critical path);
        # quarters 2+3 fully on Pool, landing well before pairs 8 and 12 ----
        p0_wq = [None] * 4
        b2q0 = b2p.tile([P, GF], FP8, tag="b2", name="p0b2_0")
        nc.vector.tensor_scalar(
            b2q0[:], p0_wt[0][:], nthr128[:], None, op0=ALU.is_le
        )
        p0_wq[0] = emit_wq_dve(p0_wt[0], b2q0, "p0wq0")
        for q in range(1, 4):
            b2 = b2p.tile([P, GF], FP8, tag="b2", name=f"p0b2{q}")
            nc.gpsimd.tensor_scalar(
                b2[:], p0_wt[q][:], nthr128[:], None, op0=ALU.is_le
            )
            wq = wqp.tile([P, GF], FP8, tag="wq", name=f"p0wq{q}")
            nc.gpsimd.scalar_tensor_tensor(
                wq[:], p0_wt[q][:], thr128[:], b2[:], op0=ALU.is_ge, op1=ALU.subtract
            )
            p0_wq[q] = wq

        # ---- Phase 2: Xq = A + B split, per k-chunk pair, resident first ----
        a_tiles = [None] * NPAIR
        b_tiles = [None] * NPAIR

        def emit_quant(j, pool=False):
            eng = nc.gpsimd if pool else nc.vector
            xt = x_tiles[j]
            ut = up.tile([P, 2 * TSH], F32, tag="u", name=f"u{j}")
            # u = x*sx + MAGIC: forces RNE to integer in the low mantissa
            # (tensor_scalar earns the DVE 2x rate; stt does not)
            eng.tensor_scalar(
                ut[:], xt[:], sx128[:], MAGIC, op0=ALU.mult, op1=ALU.add
            )
            ag = abp.tile([P, 2 * TSH], FP8, tag=f"a{j}", name=f"a{j}")
            # A = fp8_rne(u - MAGIC): ACT affine is exact fp32; the fp8
            # convert rounds the integer Xq to the e4m3 grid
            nc.scalar.activation(ag[:], ut[:], ACTF.Identity, bias=nmagic128[:])
            bg = abp.tile([P, 2 * TSH], FP8, tag=f"b{j}", name=f"b{j}")
            # B = (u - MAGIC) - A: integer in [-4, 4], exactly fp8
            eng.scalar_tensor_tensor(
                bg[:], ut[:], -MAGIC, ag[:], op0=ALU.add, op1=ALU.subtract
            )
            a_tiles[j], b_tiles[j] = ag, bg

        for j in PAIR_ORDER:
            emit_quant(j)

        # panel-1 quarters: loads behind the re-reads on the sync queue, b2
        # on Pool behind its pair work, wq on DVE right after the pairs
        p1_wt = [emit_wload(1, q, f"p1w{q}") for q in range(4)]
        p1_wq = []
        for q in range(4):
            b2 = b2p.tile([P, GF], FP8, tag="b2", name=f"p1b2{q}")
            nc.gpsimd.tensor_scalar(
                b2[:], p1_wt[q][:], nthr128[:], None, op0=ALU.is_le
            )
            p1_wq.append(emit_wq_dve(p1_wt[q], b2, f"p1wq{q}"))

        def lhsT(tiles, j, tb):
            return tiles[j][:].rearrange("p (c t) -> p c t", c=2)[
                :, :, tb * P : (tb + 1) * P
            ]

        def wq_pair(wq, jj):
            return wq[:].rearrange("p (c j) -> p c j", c=8)[:, 2 * jj : 2 * jj + 2, :]

        # ---- panel 0, pair-major in production order ----
        ps_tiles = [
            psum.tile([P, NMM], F32, tag=f"ps{tb}", name=f"p0ps{tb}")
            for tb in range(8)
        ]
        for n, j in enumerate(PAIR_ORDER):
            q, jj = j // 4, j % 4
            for tb in range(8):
                nc.tensor.matmul(
                    ps_tiles[tb][:], lhsT=lhsT(a_tiles, j, tb),
                    rhs=wq_pair(p0_wq[q], jj),
                    start=(n == 0), stop=False, perf_mode=DR,
                )
                nc.tensor.matmul(
                    ps_tiles[tb][:], lhsT=lhsT(b_tiles, j, tb),
                    rhs=wq_pair(p0_wq[q], jj),
                    start=False, stop=(n == NPAIR - 1), perf_mode=DR,
                )
        for tb in range(8):
            ot = osb.tile([P, NMM], BF16, tag="osb")
            nc.scalar.copy(ot[:], ps_tiles[tb][:])
            nc.scalar.dma_start(out[_ts(tb, P), :], ot[:])

        # ---- panels 1-7, quarter-major; b2 on Pool, wq on DVE ----
        for op_ in range(1, 8):
            if op_ == 1:
                quarters = p1_wq
            else:
                quarters = []
                for q in range(4):
                    wt = emit_wload(op_, q, f"p{op_}w{q}")
                    b2 = b2p.tile([P, GF], FP8, tag="b2")
                    nc.gpsimd.tensor_scalar(
                        b2[:], wt[:], nthr128[:], None, op0=ALU.is_le
                    )
                    quarters.append(emit_wq_dve(wt, b2, f"p{op_}wq{q}"))
            ps_tiles = [
                psum.tile([P, NMM], F32, tag=f"ps{tb}", name=f"p{op_}ps{tb}")
                for tb in range(8)
            ]
            # last panel runs token-block-major so each PSUM bank finishes
            # (and drains) as early as possible — all quarters are ready by
            # then.  Earlier panels stay quarter-major to chase the wq stream.
            if op_ == 7:
                loops = [(tb, q, jj) for tb in range(8) for q in range(4) for jj in range(4)]
            else:
                loops = [(tb, q, jj) for q in range(4) for tb in range(8) for jj in range(4)]
            for tb, q, jj in loops:
                j = q * 4 + jj
                nc.tensor.matmul(
                    ps_tiles[tb][:], lhsT=lhsT(a_tiles, j, tb),
                    rhs=wq_pair(quarters[q], jj),
                    start=(q == 0 and jj == 0), stop=False, perf_mode=DR,
                )
                nc.tensor.matmul(
                    ps_tiles[tb][:], lhsT=lhsT(b_tiles, j, tb),
                    rhs=wq_pair(quarters[q], jj),
                    start=False, stop=(q == 3 and jj == 3), perf_mode=DR,
                )
            for tb in range(8):
                ot = osb.tile([P, NMM], BF16, tag="osb")
                nc.scalar.copy(ot[:], ps_tiles[tb][:])
                nc.scalar.dma_start(out[_ts(op_ * 8 + tb, P), :], ot[:])

        # stats out for the host (nothing on-device consumes these)
        nc.gpsimd.dma_start(sout[0:1, 0:1], gsum[:])
        nc.gpsimd.dma_start(sout[0:1, 1:2], gmax[:])
        nc.gpsimd.dma_start(sout[0:1, 2:3], sx1[:])


def _build():
    nc = bacc.Bacc("TRN2", debug=False, enable_asserts=False, num_devices=NC)
    xT_ap = nc.dram_tensor("xT_shard", (I, TSH), BF16, kind="ExternalInput").ap()
    wT_ap = nc.dram_tensor("wT_full", (I, O), F32, kind="ExternalInput").ap()
    wsl_ap = nc.dram_tensor("wT_slice", (ISL, O), F32, kind="ExternalInput").ap()
    # chunked layout: row (panel*8 + tb)*128 + r, col c  <->  out[tb*128+r, panel*512+c]
    out_ap = nc.dram_tensor("out_shard", (64 * P, NMM), BF16, kind="ExternalOutput").ap()
    st_ap = nc.dram_tensor("stats_out", (1, 4), F32, kind="ExternalOutput").ap()
    with tile.TileContext(nc) as tc:
        _bitlinear(tc, out_ap, st_ap, xT_ap, wT_ap, wsl_ap)
    nc.compile()
    return nc


_NC_CACHE = None


def _get_nc():
    global _NC_CACHE
    if _NC_CACHE is None:
        _NC_CACHE = _build()
    return _NC_CACHE


def _run(x, weight, **spmd_kwargs):
    x = np.asarray(x, dtype=np.float32)
    w = np.asarray(weight, dtype=np.float32)
    assert x.shape == (T, I) and w.shape == (O, I)
    nc = _get_nc()
    wT = np.ascontiguousarray(w.T)  # [I, O]
    in_maps = [
        {
            # pure dtype cast + per-shard transpose; no stats on the host
            "xT_shard": np.ascontiguousarray(
                x[k * TSH : (k + 1) * TSH].T.astype(ml_dtypes.bfloat16)
            ),
            "wT_full": wT,
            "wT_slice": wT[k * ISL : (k + 1) * ISL],  # contiguous view
        }
        for k in range(NC)
    ]
    res = run_bass_kernel_spmd(nc, in_maps, core_ids=list(range(NC)), **spmd_kwargs)
    outs = res.results

    st0 = outs[0]["stats_out"][0]
    gsum, sx = float(st0[0]), float(st0[2])

    # replicate the reference's fp32 scalar arithmetic; nnz counted here by
    # replaying the device's exact fp32 threshold compare (host-consumed only)
    f32 = np.float32
    thr_dev = f32(f32(gsum) * f32(0.7 / float(O * I)))
    nnz = float(np.count_nonzero(np.abs(w) >= thr_dev))
    n_el = f32(float(O) * float(I))
    abs_mean = f32(f32(gsum) / n_el)
    non_zero_mean = f32(f32(f32(nnz) / n_el) + f32(1e-8))
    scale_w = f32(abs_mean / non_zero_mean)
    scale = f32(np.float64(scale_w) / np.float64(sx))

    # un-chunk each core's [8 panels][8 tb][128][512] bf16 output
    out = np.empty((T, O), dtype=np.float32)
    for k in range(NC):
        chunk = outs[k]["out_shard"].astype(np.float32).reshape(8, 8, P, NMM)
        out[k * TSH : (k + 1) * TSH] = (
            chunk.transpose(1, 2, 0, 3).reshape(TSH, O)
        )
    out *= scale
    return out, res


def kernel(x, weight):
    out, _ = _run(x, weight)
    return out


# revision 20
# speedup vs baseline: 1.0113x; 1.0113x over previous
"""BitLinear fake-quant GEMM on 8 TRN2 NeuronCores — fp8 DoubleRow edition.

Reference math:
  abs_mean  = mean(|W|);  thr = 0.7*abs_mean
  Wq        = sign(W) * (|W| >= thr)            (ternary)
  scale_w   = abs_mean / (mean(Wq != 0) + 1e-8)
  sx        = 127 / max(|X|)
  Xq        = round(X * sx)                      (integer valued, |.| <= 127)
  out       = (Xq @ Wq^T) * scale_w / sx

Sharding: data-parallel over tokens (8192/8 = 1024 columns of X^T per core);
W is replicated.  The host hands each core a PRE-TRANSPOSED bf16 x shard and
fp32 w^T, so both matmul operands have the contraction dim on partitions.

The GEMM runs on the PE in fp8e4 (e4m3) DoubleRow mode: one matmul
instruction contracts TWO 128-deep k-chunks at 0.5 cycles/row — 2x the bf16
FLOP rate.  Exactness: Xq (ints, |.|<=127) is split Xq = A + B with
A = fp8_rne(Xq) (e4m3-exact) and B = Xq - A (integer in [-4,4], fp8-exact);
Wq in {-1,0,1} is fp8-exact.  A@Wq + B@Wq accumulated in fp32 PSUM
reproduces Xq@Wq exactly (all products integers, sums < 2^24).  Net PE time
halves vs a bf16 kernel.

x is pre-converted to bf16 on the host (pure dtype cast, no stats): halves
x DMA and SBUF.  |x|-max is computed on device from the bf16 values; sx then
differs from the fp32-max reference by <= 2^-9 relative, worth ~3.6 max
output deviation against the 11.2 allowed by rel_err 2e-2.

Schedule (single 360 B/ns DMA device; every engine queue is in-order, and a
DMA whose semaphore wait is pending BLOCKS its queue's dispatch — used
deliberately below):
  sync-q: wsl x4, x pass-1 x16, collective-input hop (blocks until local
          stats land, keeping later transfers from queueing ahead of it at
          the FIFO DMA device), p0-q0 + two re-reads, collective-output hop
          (blocks until the AllGather lands), then p0-q1..q3 interleaved
          with re-reads, panels 1-7 quarters.  scalar-q: bf16 output writes.
  DVE:  x |max| pass (tensor_scalar + max accum_out at the 2x rate, |x| to a
        victim tile so the signed tiles survive), u = x*sx + MAGIC (2x),
        B = (u-MAGIC) - A, ternary wq for panels 1-7 (panel 1's interleaved
        into the last pairs).
  ACT:  |wsl| abs + sum accum_out, A = fp8(u - MAGIC), PSUM->bf16 copies.
  Pool: partition reduces/broadcasts, collective dispatch, b2 = (w<=-thr)
        for all panels, and panel-0's wq (keeps DVE free for the pairs the
        moment sx lands).
  PE:   panel 0 pair-major in production order (each arriving (A,B) pair
        feeds all 8 PSUM banks), panels 1-7 quarter-major.
  The last 3 pairs' x tiles are still resident from pass 1, so they need no
  re-read and quantize first, at sx+0.

Stats: per-core |W|-slice sum + |x|-shard max, one 2-scalar AllGather, local
reduce.  nnz (host-consumed only, for scale_w) is counted on the host by
replaying the device's exact fp32 threshold compare.  Output is written
bf16, tile-chunked; the host upcasts, scales by scale_w/sx and permutes.
"""

from contextlib import ExitStack

import numpy as np
import ml_dtypes

import concourse.bass as bass
import concourse.bass_isa as bass_isa
import concourse.tile as tile
from concourse import bacc, mybir
from concourse.bass import ts as _ts
from concourse.bass_utils import run_bass_kernel_spmd

P = 128
T, I, O = 8192, 4096, 4096  # tokens, in_features, out_features
NC = 8
TSH = T // NC  # 1024 token columns per core
ISL = I // NC  # 512 wT rows per core for stats
NMM = 512  # matmul moving free dim (one fp32 PSUM bank)
GF = 4096  # W staging tile free size (one [128, 4096] fp32 tile = 2 MB)
NPAIR = 16  # k-chunk pairs (32 chunks of 128 over I=4096)
NRES = 6  # trailing pairs whose pass-1 tiles stay resident (= xin bufs)
MAGIC = 12582912.0  # 1.5 * 2**23: fp32 round-to-nearest-even bias trick

F32 = mybir.dt.float32
BF16 = mybir.dt.bfloat16
FP8 = mybir.dt.float8e4
ALU = mybir.AluOpType
AXX = mybir.AxisListType
ACTF = mybir.ActivationFunctionType
DR = mybir.MatmulPerfMode.DoubleRow

# pass-1 load order: pairs 0..NRES-1 last, so they are the tiles still
# resident in the xin pool when sx lands — and they need quarter 0, which is
# also the first ternary quarter produced
PASS1_ORDER = list(range(NRES, NPAIR)) + list(range(NRES))
PAIR_ORDER = list(range(NPAIR))


def _bitlinear(tc, out, sout, xT, wT, wsl):
    nc = tc.nc
    with ExitStack() as ctx:
        const = ctx.enter_context(tc.tile_pool(name="const", bufs=1))
        statp = ctx.enter_context(tc.tile_pool(name="statp", bufs=1))
        dram = ctx.enter_context(tc.tile_pool(name="dram", bufs=1, space="DRAM"))
        stg = ctx.enter_context(tc.tile_pool(name="stg", bufs=4))    # f32 [128,4096]
        xin = ctx.enter_context(tc.tile_pool(name="xin", bufs=NRES))  # bf16 [128,2048]
        up = ctx.enter_context(tc.tile_pool(name="up", bufs=2))      # f32 [128,2048]
        abp = ctx.enter_context(tc.tile_pool(name="abp", bufs=1))    # fp8 [128,2048] x32
        wqp = ctx.enter_context(tc.tile_pool(name="wqp", bufs=5))    # fp8 [128,4096]
        b2p = ctx.enter_context(tc.tile_pool(name="b2p", bufs=2))    # fp8 [128,4096]
        psum = ctx.enter_context(tc.tile_pool(name="psum", bufs=1, space="PSUM"))
        osb = ctx.enter_context(tc.tile_pool(name="osb", bufs=4))    # bf16 [128,512]

        # Pool-engine consts first so they don't queue behind later Pool work
        nmagic128 = const.tile([P, 1], F32)
        nc.gpsimd.memset(nmagic128[:], -MAGIC)

        def xpair_src(j):
            # xT rows [2j*128, (2j+2)*128) as [128, 2 chunks, 1024 tokens]
            return xT[2 * j * P : (2 * j + 2) * P, :].rearrange(
                "(c p) t -> p c t", p=P
            )

        # ---- Phase 1: |W| slice sum (ACT Abs + accumulator) interleaved
        # with the x |max| pass (DVE tensor_reduce).  The wsl transfers are
        # spread through the x stream so the 2.2us-per-tile reduces pipeline
        # under the DMA stream instead of extending past it ----
        wsum_part = statp.tile([P, 4], F32)
        xmax_part = statp.tile([P, NPAIR], F32)
        x_tiles = [None] * NPAIR

        def emit_wsl(c):
            wt = stg.tile([P, GF], F32, tag="stg", name=f"wsl{c}")
            nc.sync.dma_start(wt[:], wsl[_ts(c, P), :])
            # in-place |w|; accum_out gives the per-partition row sum free
            nc.scalar.activation(
                wt[:], wt[:], ACTF.Abs, accum_out=wsum_part[:, c : c + 1]
            )

        def emit_x1(j):
            xt = xin.tile([P, 2 * TSH], BF16, tag="xin", name=f"x1_{j}")
            nc.sync.dma_start(
                xt[:].rearrange("p (c t) -> p c t", c=2), xpair_src(j)
            )
            nc.vector.tensor_reduce(
                xmax_part[:, j : j + 1], xt[:], axis=AXX.X, op=ALU.max,
                apply_absolute_value=True,
            )
            x_tiles[j] = xt

        for n, j in enumerate(PASS1_ORDER):
            if n in (2, 6, 10, 14):
                emit_wsl((n - 2) // 4)
            emit_x1(j)

        wsum_c = statp.tile([P, 1], F32)
        nc.vector.tensor_reduce(wsum_c[:], wsum_part[:], axis=AXX.X, op=ALU.add)
        xmax_c = statp.tile([P, 1], F32)
        nc.vector.tensor_reduce(xmax_c[:], xmax_part[:], axis=AXX.X, op=ALU.max)
        wsum_a = statp.tile([P, 1], F32)
        nc.gpsimd.partition_all_reduce(
            wsum_a[:], wsum_c[:], channels=P, reduce_op=bass_isa.ReduceOp.add
        )
        xmax_a = statp.tile([P, 1], F32)
        nc.gpsimd.partition_all_reduce(
            xmax_a[:], xmax_c[:], channels=P, reduce_op=bass_isa.ReduceOp.max
        )

        # ---- one tiny AllGather of [wsum, xmax]; reduce locally.  Both DRAM
        # hops ride the sync queue: a waiting DMA blocks the queue's
        # dispatch, so later bulk transfers cannot queue ahead of the hop at
        # the FIFO DMA device — each hop fires the moment its data is ready --
        loc = statp.tile([1, 2], F32)
        nc.vector.tensor_copy(loc[0:1, 0:1], wsum_a[0:1, 0:1])
        nc.vector.tensor_copy(loc[0:1, 1:2], xmax_a[0:1, 0:1])
        cin = dram.tile([1, 2], F32)
        cout = dram.tile([1, 2 * NC], F32)
        nc.sync.dma_start(cin[:], loc[:])
        nc.gpsimd.collective_compute(
            "AllGather", ALU.bypass, replica_groups=[list(range(NC))],
            ins=[cin.opt()], outs=[cout.opt()],
        )

        def emit_wload(op_, q, nm):
            wt = stg.tile([P, GF], F32, tag="stg", name=nm)
            src = wT[
                q * 1024 : (q + 1) * 1024, _ts(op_, NMM)
            ].rearrange("(c p) j -> p c j", p=P)
            nc.sync.dma_start(wt[:].rearrange("p (c j) -> p c j", c=8), src)
            return wt

        def emit_re(j):
            xt = xin.tile([P, 2 * TSH], BF16, tag="xin", name=f"xre{j}")
            nc.sync.dma_start(
                xt[:].rearrange("p (c t) -> p c t", c=2), xpair_src(j)
            )
            x_tiles[j] = xt

        # p0-q0 + two re-reads slot in ahead of the collective's output hop
        p0_wt = [None] * 4
        p0_wt[0] = emit_wload(0, 0, "p0w0")
        emit_re(6)
        emit_re(7)

        gg = statp.tile([1, 2 * NC], F32)
        nc.sync.dma_start(gg[:], cout[:])

        p0_wt[1] = emit_wload(0, 1, "p0w1")
        emit_re(8)
        emit_re(9)
        p0_wt[2] = emit_wload(0, 2, "p0w2")
        emit_re(10)
        emit_re(11)
        p0_wt[3] = emit_wload(0, 3, "p0w3")
        for j in range(12, NPAIR):
            emit_re(j)

        gg3 = gg[:].rearrange("a (r k) -> a r k", k=2)
        gsum = statp.tile([1, 1], F32)
        nc.vector.tensor_reduce(gsum[:], gg3[:, :, 0:1], axis=AXX.XY, op=ALU.add)
        gmax = statp.tile([1, 1], F32)
        nc.vector.tensor_reduce(gmax[:], gg3[:, :, 1:2], axis=AXX.XY, op=ALU.max)

        thr1 = statp.tile([1, 1], F32)
        nc.vector.tensor_scalar(thr1[:], gsum[:], 0.7 / float(O * I), None, op0=ALU.mult)
        nthr1 = statp.tile([1, 1], F32)
        nc.vector.tensor_scalar(nthr1[:], thr1[:], -1.0, None, op0=ALU.mult)
        thr128 = const.tile([P, 1], F32)
        nc.gpsimd.partition_broadcast(thr128[:], thr1[:])
        nthr128 = const.tile([P, 1], F32)
        nc.gpsimd.partition_broadcast(nthr128[:], nthr1[:])

        gmax_c = statp.tile([1, 1], F32)
        nc.vector.tensor_scalar(gmax_c[:], gmax[:], 1e-12, None, op0=ALU.max)
        rec1 = statp.tile([1, 1], F32)
        nc.vector.reciprocal(rec1[:], gmax_c[:])
        sx1 = statp.tile([1, 1], F32)
        nc.vector.tensor_scalar(sx1[:], rec1[:], 127.0, None, op0=ALU.mult)
        sx128 = const.tile([P, 1], F32)
        nc.gpsimd.partition_broadcast(sx128[:], sx1[:])

        def emit_wq_dve(wt, b2, nm):
            wq = wqp.tile([P, GF], FP8, tag="wq", name=nm)
            nc.vector.scalar_tensor_tensor(
                wq[:], wt[:], thr128[:], b2[:], op0=ALU.is_ge, op1=ALU.subtract
            )
            return wq

        # ---- panel-0 quant: quarters 0+1 ternary on DVE ahead of the pair
        # stream (q0's b2 too — no Pool dependency on the critical path);
        # quarters 2+3 fully on Pool, landing well before pairs 8 and 12 ----
        p0_wq = [None] * 4
        b2q0 = b2p.tile([P, GF], FP8, tag="b2", name="p0b2_0")
        nc.vector.tensor_scalar(
            b2q0[:], p0_wt[0][:], nthr128[:], None, op0=ALU.is_le
        )
        p0_wq[0] = emit_wq_dve(p0_wt[0], b2q0, "p0wq0")
        p0_b2 = [b2q0]
        for q in range(1, 4):
            b2 = b2p.tile([P, GF], FP8, tag="b2", name=f"p0b2{q}")
            nc.gpsimd.tensor_scalar(
                b2[:], p0_wt[q][:], nthr128[:], None, op0=ALU.is_le
            )
            p0_b2.append(b2)
        p0_wq[1] = emit_wq_dve(p0_wt[1], p0_b2[1], "p0wq1")

        # ---- Phase 2: Xq = A + B split, per k-chunk pair, resident first ----
        a_tiles = [None] * NPAIR
        b_tiles = [None] * NPAIR

        def emit_quant(j, pool=False):
            eng = nc.gpsimd if pool else nc.vector
            xt = x_tiles[j]
            ut = up.tile([P, 2 * TSH], F32, tag="u", name=f"u{j}")
            # u = x*sx + MAGIC: forces RNE to integer in the low mantissa
            # (tensor_scalar earns the DVE 2x rate; stt does not)
            eng.tensor_scalar(
                ut[:], xt[:], sx128[:], MAGIC, op0=ALU.mult, op1=ALU.add
            )
            ag = abp.tile([P, 2 * TSH], FP8, tag=f"a{j}", name=f"a{j}")
            # A = fp8_rne(u - MAGIC): ACT affine is exact fp32; the fp8
            # convert rounds the integer Xq to the e4m3 grid
            nc.scalar.activation(ag[:], ut[:], ACTF.Identity, bias=nmagic128[:])
            bg = abp.tile([P, 2 * TSH], FP8, tag=f"b{j}", name=f"b{j}")
            # B = (u - MAGIC) - A: integer in [-4, 4], exactly fp8
            eng.scalar_tensor_tensor(
                bg[:], ut[:], -MAGIC, ag[:], op0=ALU.add, op1=ALU.subtract
            )
            a_tiles[j], b_tiles[j] = ag, bg

        # q2/q3 ternaries (DVE stt — not a valid Pool opcode) interleave
        # into the pair stream just ahead of the matmuls that need them
        for j in PAIR_ORDER[:6]:
            emit_quant(j)
        p0_wq[2] = emit_wq_dve(p0_wt[2], p0_b2[2], "p0wq2")
        for j in PAIR_ORDER[6:10]:
            emit_quant(j)
        p0_wq[3] = emit_wq_dve(p0_wt[3], p0_b2[3], "p0wq3")
        for j in PAIR_ORDER[10:]:
            emit_quant(j)

        # panel-1 quarters: loads behind the re-reads on the sync queue, b2
        # on Pool behind its pair work, wq on DVE right after the pairs
        p1_wt = [emit_wload(1, q, f"p1w{q}") for q in range(4)]
        p1_wq = []
        for q in range(4):
            b2 = b2p.tile([P, GF], FP8, tag="b2", name=f"p1b2{q}")
            nc.gpsimd.tensor_scalar(
                b2[:], p1_wt[q][:], nthr128[:], None, op0=ALU.is_le
            )
            p1_wq.append(emit_wq_dve(p1_wt[q], b2, f"p1wq{q}"))

        def lhsT(tiles, j, tb):
            return tiles[j][:].rearrange("p (c t) -> p c t", c=2)[
                :, :, tb * P : (tb + 1) * P
            ]

        def wq_pair(wq, jj):
            return wq[:].rearrange("p (c j) -> p c j", c=8)[:, 2 * jj : 2 * jj + 2, :]

        # ---- panel 0, pair-major in production order ----
        ps_tiles = [
            psum.tile([P, NMM], F32, tag=f"ps{tb}", name=f"p0ps{tb}")
            for tb in range(8)
        ]
        for n, j in enumerate(PAIR_ORDER):
            q, jj = j // 4, j % 4
            for tb in range(8):
                nc.tensor.matmul(
                    ps_tiles[tb][:], lhsT=lhsT(a_tiles, j, tb),
                    rhs=wq_pair(p0_wq[q], jj),
                    start=(n == 0), stop=False, perf_mode=DR,
                )
                nc.tensor.matmul(
                    ps_tiles[tb][:], lhsT=lhsT(b_tiles, j, tb),
                    rhs=wq_pair(p0_wq[q], jj),
                    start=False, stop=(n == NPAIR - 1), perf_mode=DR,
                )
        for tb in range(8):
            ot = osb.tile([P, NMM], BF16, tag="osb")
            nc.scalar.copy(ot[:], ps_tiles[tb][:])
            nc.scalar.dma_start(out[_ts(tb, P), :], ot[:])

        # ---- panels 1-7, quarter-major; b2 on Pool, wq on DVE ----
        for op_ in range(1, 8):
            if op_ == 1:
                quarters = p1_wq
            else:
                quarters = []
                for q in range(4):
                    wt = emit_wload(op_, q, f"p{op_}w{q}")
                    b2 = b2p.tile([P, GF], FP8, tag="b2")
                    nc.gpsimd.tensor_scalar(
                        b2[:], wt[:], nthr128[:], None, op0=ALU.is_le
                    )
                    quarters.append(emit_wq_dve(wt, b2, f"p{op_}wq{q}"))
            ps_tiles = [
                psum.tile([P, NMM], F32, tag=f"ps{tb}", name=f"p{op_}ps{tb}")
                for tb in range(8)
            ]
            # last panel runs token-block-major so each PSUM bank finishes
            # (and drains) as early as possible — all quarters are ready by
            # then.  Earlier panels stay quarter-major to chase the wq stream.
            if op_ == 7:
                loops = [(tb, q, jj) for tb in range(8) for q in range(4) for jj in range(4)]
            else:
                loops = [(tb, q, jj) for q in range(4) for tb in range(8) for jj in range(4)]
            for tb, q, jj in loops:
                j = q * 4 + jj
                nc.tensor.matmul(
                    ps_tiles[tb][:], lhsT=lhsT(a_tiles, j, tb),
                    rhs=wq_pair(quarters[q], jj),
                    start=(q == 0 and jj == 0), stop=False, perf_mode=DR,
                )
                nc.tensor.matmul(
                    ps_tiles[tb][:], lhsT=lhsT(b_tiles, j, tb),
                    rhs=wq_pair(quarters[q], jj),
                    start=False, stop=(q == 3 and jj == 3), perf_mode=DR,
                )
            for tb in range(8):
                ot = osb.tile([P, NMM], BF16, tag="osb")
                nc.scalar.copy(ot[:], ps_tiles[tb][:])
                nc.scalar.dma_start(out[_ts(op_ * 8 + tb, P), :], ot[:])

        # stats out for the host (nothing on-device consumes these)
        nc.gpsimd.dma_start(sout[0:1, 0:1], gsum[:])
        nc.gpsimd.dma_start(sout[0:1, 1:2], gmax[:])
        nc.gpsimd.dma_start(sout[0:1, 2:3], sx1[:])


def _build():
    nc = bacc.Bacc("TRN2", debug=False, enable_asserts=False, num_devices=NC)
    xT_ap = nc.dram_tensor("xT_shard", (I, TSH), BF16, kind="ExternalInput").ap()
    wT_ap = nc.dram_tensor("wT_full", (I, O), F32, kind="ExternalInput").ap()
    wsl_ap = nc.dram_tensor("wT_slice", (ISL, O), F32, kind="ExternalInput").ap()
    # chunked layout: row (panel*8 + tb)*128 + r, col c  <->  out[tb*128+r, panel*512+c]
    out_ap = nc.dram_tensor("out_shard", (64 * P, NMM), BF16, kind="ExternalOutput").ap()
    st_ap = nc.dram_tensor("stats_out", (1, 4), F32, kind="ExternalOutput").ap()
    with tile.TileContext(nc) as tc:
        _bitlinear(tc, out_ap, st_ap, xT_ap, wT_ap, wsl_ap)
    nc.compile()
    return nc


_NC_CACHE = None


def _get_nc():
    global _NC_CACHE
    if _NC_CACHE is None:
        _NC_CACHE = _build()
    return _NC_CACHE


def _run(x, weight, **spmd_kwargs):
    x = np.asarray(x, dtype=np.float32)
    w = np.asarray(weight, dtype=np.float32)
    assert x.shape == (T, I) and w.shape == (O, I)
    nc = _get_nc()
    wT = np.ascontiguousarray(w.T)  # [I, O]
    in_maps = [
        {
            # pure dtype cast + per-shard transpose; no stats on the host
            "xT_shard": np.ascontiguousarray(
                x[k * TSH : (k + 1) * TSH].T.astype(ml_dtypes.bfloat16)
            ),
            "wT_full": wT,
            "wT_slice": wT[k * ISL : (k + 1) * ISL],  # contiguous view
        }
        for k in range(NC)
    ]
    res = run_bass_kernel_spmd(nc, in_maps, core_ids=list(range(NC)), **spmd_kwargs)
    outs = res.results

    st0 = outs[0]["stats_out"][0]
    gsum, sx = float(st0[0]), float(st0[2])

    # replicate the reference's fp32 scalar arithmetic; nnz counted here by
    # replaying the device's exact fp32 threshold compare (host-consumed only)
    f32 = np.float32
    thr_dev = f32(f32(gsum) * f32(0.7 / float(O * I)))
    nnz = float(np.count_nonzero(np.abs(w) >= thr_dev))
    n_el = f32(float(O) * float(I))
    abs_mean = f32(f32(gsum) / n_el)
    non_zero_mean = f32(f32(f32(nnz) / n_el) + f32(1e-8))
    scale_w = f32(abs_mean / non_zero_mean)
    scale = f32(np.float64(scale_w) / np.float64(sx))

    # un-chunk each core's [8 panels][8 tb][128][512] bf16 output
    out = np.empty((T, O), dtype=np.float32)
    for k in range(NC):
        chunk = outs[k]["out_shard"].astype(np.float32).reshape(8, 8, P, NMM)
        out[k * TSH : (k + 1) * TSH] = (
            chunk.transpose(1, 2, 0, 3).reshape(TSH, O)
        )
    out *= scale
    return out, res


def kernel(x, weight):
    out, _ = _run(x, weight)
    return out


# revision 21
# speedup vs baseline: 1.0252x; 1.0138x over previous
"""BitLinear fake-quant GEMM on 8 TRN2 NeuronCores — fp8 DoubleRow edition.

Reference math:
  abs_mean  = mean(|W|);  thr = 0.7*abs_mean
  Wq        = sign(W) * (|W| >= thr)            (ternary)
  scale_w   = abs_mean / (mean(Wq != 0) + 1e-8)
  sx        = 127 / max(|X|)
  Xq        = round(X * sx)                      (integer valued, |.| <= 127)
  out       = (Xq @ Wq^T) * scale_w / sx

Sharding: data-parallel over tokens (8192/8 = 1024 columns of X^T per core);
W is replicated.  The host hands each core a PRE-TRANSPOSED bf16 x shard and
fp32 w^T, so both matmul operands have the contraction dim on partitions.

The GEMM runs on the PE in fp8e4 (e4m3) DoubleRow mode: one matmul
instruction contracts TWO 128-deep k-chunks at 0.5 cycles/row — 2x the bf16
FLOP rate.  Exactness: Xq (ints, |.|<=127) is split Xq = A + B with
A = fp8_rne(Xq) (e4m3-exact) and B = Xq - A (integer in [-4,4], fp8-exact);
Wq in {-1,0,1} is fp8-exact.  A@Wq + B@Wq accumulated in fp32 PSUM
reproduces Xq@Wq exactly (all products integers, sums < 2^24).  Net PE time
halves vs a bf16 kernel.

x is pre-converted to bf16 on the host (pure dtype cast, no stats): halves
x DMA and SBUF.  |x|-max is computed on device from the bf16 values; sx then
differs from the fp32-max reference by <= 2^-9 relative, worth ~3.6 max
output deviation against the 11.2 allowed by rel_err 2e-2.

Schedule (single 360 B/ns DMA device; every engine queue is in-order, and a
DMA whose semaphore wait is pending BLOCKS its queue's dispatch — used
deliberately below):
  sync-q: wsl x4, x pass-1 x16, collective-input hop (blocks until local
          stats land, keeping later transfers from queueing ahead of it at
          the FIFO DMA device), p0-q0 + two re-reads, collective-output hop
          (blocks until the AllGather lands), then p0-q1..q3 interleaved
          with re-reads, panels 1-7 quarters.  scalar-q: bf16 output writes.
  DVE:  x |max| pass (tensor_scalar + max accum_out at the 2x rate, |x| to a
        victim tile so the signed tiles survive), u = x*sx + MAGIC (2x),
        B = (u-MAGIC) - A, ternary wq for panels 1-7 (panel 1's interleaved
        into the last pairs).
  ACT:  |wsl| abs + sum accum_out, A = fp8(u - MAGIC), PSUM->bf16 copies.
  Pool: partition reduces/broadcasts, collective dispatch, b2 = (w<=-thr)
        for all panels, and panel-0's wq (keeps DVE free for the pairs the
        moment sx lands).
  PE:   panel 0 pair-major in production order (each arriving (A,B) pair
        feeds all 8 PSUM banks), panels 1-7 quarter-major.
  The last 3 pairs' x tiles are still resident from pass 1, so they need no
  re-read and quantize first, at sx+0.

Stats: per-core |W|-slice sum + |x|-shard max, one 2-scalar AllGather, local
reduce.  nnz (host-consumed only, for scale_w) is counted on the host by
replaying the device's exact fp32 threshold compare.  Output is written
bf16, tile-chunked; the host upcasts, scales by scale_w/sx and permutes.
"""

from contextlib import ExitStack

import numpy as np
import ml_dtypes

import concourse.bass as bass
import concourse.bass_isa as bass_isa
import concourse.tile as tile
from concourse import bacc, mybir
from concourse.bass import ts as _ts
from concourse.bass_utils import run_bass_kernel_spmd

P = 128
T, I, O = 8192, 4096, 4096  # tokens, in_features, out_features
NC = 8
TSH = T // NC  # 1024 token columns per core
ISL = I // NC  # 512 wT rows per core for stats
NMM = 512  # matmul moving free dim (one fp32 PSUM bank)
GF = 4096  # W staging tile free size (one [128, 4096] fp32 tile = 2 MB)
NPAIR = 16  # k-chunk pairs (32 chunks of 128 over I=4096)
NRES = 6  # trailing pairs whose pass-1 tiles stay resident (= xin bufs)
MAGIC = 12582912.0  # 1.5 * 2**23: fp32 round-to-nearest-even bias trick

F32 = mybir.dt.float32
BF16 = mybir.dt.bfloat16
FP8 = mybir.dt.float8e4
ALU = mybir.AluOpType
AXX = mybir.AxisListType
ACTF = mybir.ActivationFunctionType
DR = mybir.MatmulPerfMode.DoubleRow

# pass-1 load order: pairs 0..NRES-1 last, so they are the tiles still
# resident in the xin pool when sx lands — and they need quarter 0, which is
# also the first ternary quarter produced
PASS1_ORDER = list(range(NRES, NPAIR)) + list(range(NRES))
PAIR_ORDER = list(range(NPAIR))


def _bitlinear(tc, out, sout, xT, wT, wsl):
    nc = tc.nc
    with ExitStack() as ctx:
        const = ctx.enter_context(tc.tile_pool(name="const", bufs=1))
        statp = ctx.enter_context(tc.tile_pool(name="statp", bufs=1))
        dram = ctx.enter_context(tc.tile_pool(name="dram", bufs=1, space="DRAM"))
        stg = ctx.enter_context(tc.tile_pool(name="stg", bufs=4))    # f32 [128,4096]
        xin = ctx.enter_context(tc.tile_pool(name="xin", bufs=NRES))  # bf16 [128,2048]
        up = ctx.enter_context(tc.tile_pool(name="up", bufs=2))      # f32 [128,2048]
        abp = ctx.enter_context(tc.tile_pool(name="abp", bufs=1))    # fp8 [128,2048] x32
        wqp = ctx.enter_context(tc.tile_pool(name="wqp", bufs=5))    # fp8 [128,4096]
        b2p = ctx.enter_context(tc.tile_pool(name="b2p", bufs=2))    # fp8 [128,4096]
        psum = ctx.enter_context(tc.tile_pool(name="psum", bufs=1, space="PSUM"))
        osb = ctx.enter_context(tc.tile_pool(name="osb", bufs=4))    # bf16 [128,512]

        # Pool-engine consts first so they don't queue behind later Pool work
        nmagic128 = const.tile([P, 1], F32)
        nc.gpsimd.memset(nmagic128[:], -MAGIC)

        def xpair_src(j):
            # xT rows [2j*128, (2j+2)*128) as [128, 2 chunks, 1024 tokens]
            return xT[2 * j * P : (2 * j + 2) * P, :].rearrange(
                "(c p) t -> p c t", p=P
            )

        # ---- Phase 1: |W| slice sum (ACT Abs + accumulator) interleaved
        # with the x |max| pass (DVE tensor_reduce).  The wsl transfers are
        # spread through the x stream so the 2.2us-per-tile reduces pipeline
        # under the DMA stream instead of extending past it ----
        wsum_part = statp.tile([P, 4], F32)
        xmax_part = statp.tile([P, NPAIR], F32)
        x_tiles = [None] * NPAIR

        def emit_wsl(c):
            wt = stg.tile([P, GF], F32, tag="stg", name=f"wsl{c}")
            nc.sync.dma_start(wt[:], wsl[_ts(c, P), :])
            # in-place |w|; accum_out gives the per-partition row sum free
            nc.scalar.activation(
                wt[:], wt[:], ACTF.Abs, accum_out=wsum_part[:, c : c + 1]
            )

        def emit_x1(j):
            xt = xin.tile([P, 2 * TSH], BF16, tag="xin", name=f"x1_{j}")
            nc.sync.dma_start(
                xt[:].rearrange("p (c t) -> p c t", c=2), xpair_src(j)
            )
            nc.vector.tensor_reduce(
                xmax_part[:, j : j + 1], xt[:], axis=AXX.X, op=ALU.max,
                apply_absolute_value=True,
            )
            x_tiles[j] = xt

        for n, j in enumerate(PASS1_ORDER):
            if n in (2, 6, 10, 14):
                emit_wsl((n - 2) // 4)
            emit_x1(j)

        wsum_c = statp.tile([P, 1], F32)
        nc.vector.tensor_reduce(wsum_c[:], wsum_part[:], axis=AXX.X, op=ALU.add)
        xmax_c = statp.tile([P, 1], F32)
        nc.vector.tensor_reduce(xmax_c[:], xmax_part[:], axis=AXX.X, op=ALU.max)
        wsum_a = statp.tile([P, 1], F32)
        nc.gpsimd.partition_all_reduce(
            wsum_a[:], wsum_c[:], channels=P, reduce_op=bass_isa.ReduceOp.add
        )
        xmax_a = statp.tile([P, 1], F32)
        nc.gpsimd.partition_all_reduce(
            xmax_a[:], xmax_c[:], channels=P, reduce_op=bass_isa.ReduceOp.max
        )

        # ---- one tiny AllGather of [wsum, xmax]; reduce locally.  Both DRAM
        # hops ride the sync queue: a waiting DMA blocks the queue's
        # dispatch, so later bulk transfers cannot queue ahead of the hop at
        # the FIFO DMA device — each hop fires the moment its data is ready --
        loc = statp.tile([1, 2], F32)
        nc.vector.tensor_copy(loc[0:1, 0:1], wsum_a[0:1, 0:1])
        nc.vector.tensor_copy(loc[0:1, 1:2], xmax_a[0:1, 0:1])
        cin = dram.tile([1, 2], F32)
        cout = dram.tile([1, 2 * NC], F32)
        nc.sync.dma_start(cin[:], loc[:])
        nc.gpsimd.collective_compute(
            "AllGather", ALU.bypass, replica_groups=[list(range(NC))],
            ins=[cin.opt()], outs=[cout.opt()],
        )

        def emit_wload(op_, q, nm):
            wt = stg.tile([P, GF], F32, tag="stg", name=nm)
            src = wT[
                q * 1024 : (q + 1) * 1024, _ts(op_, NMM)
            ].rearrange("(c p) j -> p c j", p=P)
            nc.sync.dma_start(wt[:].rearrange("p (c j) -> p c j", c=8), src)
            return wt

        def emit_re(j):
            xt = xin.tile([P, 2 * TSH], BF16, tag="xin", name=f"xre{j}")
            nc.sync.dma_start(
                xt[:].rearrange("p (c t) -> p c t", c=2), xpair_src(j)
            )
            x_tiles[j] = xt

        # p0-q0 + two re-reads slot in ahead of the collective's output hop
        p0_wt = [None] * 4
        p0_wt[0] = emit_wload(0, 0, "p0w0")
        emit_re(6)
        emit_re(7)

        gg = statp.tile([1, 2 * NC], F32)
        nc.sync.dma_start(gg[:], cout[:])

        p0_wt[1] = emit_wload(0, 1, "p0w1")
        emit_re(8)
        emit_re(9)
        p0_wt[2] = emit_wload(0, 2, "p0w2")
        emit_re(10)
        emit_re(11)
        p0_wt[3] = emit_wload(0, 3, "p0w3")
        for j in range(12, NPAIR):
            emit_re(j)

        gg3 = gg[:].rearrange("a (r k) -> a r k", k=2)
        gsum = statp.tile([1, 1], F32)
        nc.vector.tensor_reduce(gsum[:], gg3[:, :, 0:1], axis=AXX.XY, op=ALU.add)
        gmax = statp.tile([1, 1], F32)
        nc.vector.tensor_reduce(gmax[:], gg3[:, :, 1:2], axis=AXX.XY, op=ALU.max)

        thr1 = statp.tile([1, 1], F32)
        nc.vector.tensor_scalar(thr1[:], gsum[:], 0.7 / float(O * I), None, op0=ALU.mult)
        nthr1 = statp.tile([1, 1], F32)
        nc.vector.tensor_scalar(nthr1[:], thr1[:], -1.0, None, op0=ALU.mult)
        thr128 = const.tile([P, 1], F32)
        nc.gpsimd.partition_broadcast(thr128[:], thr1[:])
        nthr128 = const.tile([P, 1], F32)
        nc.gpsimd.partition_broadcast(nthr128[:], nthr1[:])

        gmax_c = statp.tile([1, 1], F32)
        nc.vector.tensor_scalar(gmax_c[:], gmax[:], 1e-12, None, op0=ALU.max)
        rec1 = statp.tile([1, 1], F32)
        nc.vector.reciprocal(rec1[:], gmax_c[:])
        sx1 = statp.tile([1, 1], F32)
        nc.vector.tensor_scalar(sx1[:], rec1[:], 127.0, None, op0=ALU.mult)
        sx128 = const.tile([P, 1], F32)
        nc.gpsimd.partition_broadcast(sx128[:], sx1[:])

        def emit_wq_dve(wt, b2, nm):
            wq = wqp.tile([P, GF], FP8, tag="wq", name=nm)
            nc.vector.scalar_tensor_tensor(
                wq[:], wt[:], thr128[:], b2[:], op0=ALU.is_ge, op1=ALU.subtract
            )
            return wq

        # ---- panel-0 quant: quarters 0+1 ternary on DVE ahead of the pair
        # stream (q0's b2 too — no Pool dependency on the critical path);
        # quarters 2+3 fully on Pool, landing well before pairs 8 and 12 ----
        p0_wq = [None] * 4
        b2q0 = b2p.tile([P, GF], FP8, tag="b2", name="p0b2_0")
        nc.vector.tensor_scalar(
            b2q0[:], p0_wt[0][:], nthr128[:], None, op0=ALU.is_le
        )
        p0_wq[0] = emit_wq_dve(p0_wt[0], b2q0, "p0wq0")
        p0_b2 = [b2q0]
        for q in range(1, 4):
            b2 = b2p.tile([P, GF], FP8, tag="b2", name=f"p0b2{q}")
            nc.gpsimd.tensor_scalar(
                b2[:], p0_wt[q][:], nthr128[:], None, op0=ALU.is_le
            )
            p0_b2.append(b2)
        p0_wq[1] = emit_wq_dve(p0_wt[1], p0_b2[1], "p0wq1")

        # ---- Phase 2: Xq = A + B split, per k-chunk pair, resident first ----
        a_tiles = [None] * NPAIR
        b_tiles = [None] * NPAIR

        def emit_quant(j, pool=False):
            ueng = nc.gpsimd if pool else nc.vector
            eng = nc.vector
            xt = x_tiles[j]
            ut = up.tile([P, 2 * TSH], F32, tag="u", name=f"u{j}")
            # u = x*sx + MAGIC: forces RNE to integer in the low mantissa
            # (tensor_scalar earns the DVE 2x rate; stt does not)
            ueng.tensor_scalar(
                ut[:], xt[:], sx128[:], MAGIC, op0=ALU.mult, op1=ALU.add
            )
            ag = abp.tile([P, 2 * TSH], FP8, tag=f"a{j}", name=f"a{j}")
            # A = fp8_rne(u - MAGIC): ACT affine is exact fp32; the fp8
            # convert rounds the integer Xq to the e4m3 grid
            nc.scalar.activation(ag[:], ut[:], ACTF.Identity, bias=nmagic128[:])
            bg = abp.tile([P, 2 * TSH], FP8, tag=f"b{j}", name=f"b{j}")
            # B = (u - MAGIC) - A: integer in [-4, 4], exactly fp8
            eng.scalar_tensor_tensor(
                bg[:], ut[:], -MAGIC, ag[:], op0=ALU.add, op1=ALU.subtract
            )
            a_tiles[j], b_tiles[j] = ag, bg

        # q2/q3 ternaries (DVE stt — not a valid Pool opcode) interleave
        # into the pair stream just ahead of the matmuls that need them
        for j in PAIR_ORDER[:6]:
            emit_quant(j, pool=(j % 2 == 1))
        p0_wq[2] = emit_wq_dve(p0_wt[2], p0_b2[2], "p0wq2")
        for j in PAIR_ORDER[6:10]:
            emit_quant(j, pool=(j % 2 == 1))
        p0_wq[3] = emit_wq_dve(p0_wt[3], p0_b2[3], "p0wq3")
        for j in PAIR_ORDER[10:]:
            emit_quant(j, pool=(j % 2 == 1))

        # panel-1 quarters: loads behind the re-reads on the sync queue, b2
        # on Pool behind its pair work, wq on DVE right after the pairs
        p1_wt = [emit_wload(1, q, f"p1w{q}") for q in range(4)]
        p1_wq = []
        for q in range(4):
            b2 = b2p.tile([P, GF], FP8, tag="b2", name=f"p1b2{q}")
            nc.gpsimd.tensor_scalar(
                b2[:], p1_wt[q][:], nthr128[:], None, op0=ALU.is_le
            )
            p1_wq.append(emit_wq_dve(p1_wt[q], b2, f"p1wq{q}"))

        def lhsT(tiles, j, tb):
            return tiles[j][:].rearrange("p (c t) -> p c t", c=2)[
                :, :, tb * P : (tb + 1) * P
            ]

        def wq_pair(wq, jj):
            return wq[:].rearrange("p (c j) -> p c j", c=8)[:, 2 * jj : 2 * jj + 2, :]

        # ---- panel 0, pair-major in production order ----
        ps_tiles = [
            psum.tile([P, NMM], F32, tag=f"ps{tb}", name=f"p0ps{tb}")
            for tb in range(8)
        ]
        for n, j in enumerate(PAIR_ORDER):
            q, jj = j // 4, j % 4
            for tb in range(8):
                nc.tensor.matmul(
                    ps_tiles[tb][:], lhsT=lhsT(a_tiles, j, tb),
                    rhs=wq_pair(p0_wq[q], jj),
                    start=(n == 0), stop=False, perf_mode=DR,
                )
                nc.tensor.matmul(
                    ps_tiles[tb][:], lhsT=lhsT(b_tiles, j, tb),
                    rhs=wq_pair(p0_wq[q], jj),
                    start=False, stop=(n == NPAIR - 1), perf_mode=DR,
                )
        for tb in range(8):
            ot = osb.tile([P, NMM], BF16, tag="osb")
            nc.scalar.copy(ot[:], ps_tiles[tb][:])
            nc.scalar.dma_start(out[_ts(tb, P), :], ot[:])

        # ---- panels 1-7, quarter-major; b2 on Pool, wq on DVE ----
        for op_ in range(1, 8):
            if op_ == 1:
                quarters = p1_wq
            else:
                quarters = []
                for q in range(4):
                    wt = emit_wload(op_, q, f"p{op_}w{q}")
                    b2 = b2p.tile([P, GF], FP8, tag="b2")
                    nc.gpsimd.tensor_scalar(
                        b2[:], wt[:], nthr128[:], None, op0=ALU.is_le
                    )
                    quarters.append(emit_wq_dve(wt, b2, f"p{op_}wq{q}"))
            ps_tiles = [
                psum.tile([P, NMM], F32, tag=f"ps{tb}", name=f"p{op_}ps{tb}")
                for tb in range(8)
            ]
            # last panel runs token-block-major so each PSUM bank finishes
            # (and drains) as early as possible — all quarters are ready by
            # then.  Earlier panels stay quarter-major to chase the wq stream.
            if op_ == 7:
                loops = [(tb, q, jj) for tb in range(8) for q in range(4) for jj in range(4)]
            else:
                loops = [(tb, q, jj) for q in range(4) for tb in range(8) for jj in range(4)]
            for tb, q, jj in loops:
                j = q * 4 + jj
                nc.tensor.matmul(
                    ps_tiles[tb][:], lhsT=lhsT(a_tiles, j, tb),
                    rhs=wq_pair(quarters[q], jj),
                    start=(q == 0 and jj == 0), stop=False, perf_mode=DR,
                )
                nc.tensor.matmul(
                    ps_tiles[tb][:], lhsT=lhsT(b_tiles, j, tb),
                    rhs=wq_pair(quarters[q], jj),
                    start=False, stop=(q == 3 and jj == 3), perf_mode=DR,
                )
            for tb in range(8):
                ot = osb.tile([P, NMM], BF16, tag="osb")
                nc.scalar.copy(ot[:], ps_tiles[tb][:])
                nc.scalar.dma_start(out[_ts(op_ * 8 + tb, P), :], ot[:])

        # stats out for the host (nothing on-device consumes these)
        nc.gpsimd.dma_start(sout[0:1, 0:1], gsum[:])
        nc.gpsimd.dma_start(sout[0:1, 1:2], gmax[:])
        nc.gpsimd.dma_start(sout[0:1, 2:3], sx1[:])


def _build():
    nc = bacc.Bacc("TRN2", debug=False, enable_asserts=False, num_devices=NC)
    xT_ap = nc.dram_tensor("xT_shard", (I, TSH), BF16, kind="ExternalInput").ap()
    wT_ap = nc.dram_tensor("wT_full", (I, O), F32, kind="ExternalInput").ap()
    wsl_ap = nc.dram_tensor("wT_slice", (ISL, O), F32, kind="ExternalInput").ap()
    # chunked layout: row (panel*8 + tb)*128 + r, col c  <->  out[tb*128+r, panel*512+c]
    out_ap = nc.dram_tensor("out_shard", (64 * P, NMM), BF16, kind="ExternalOutput").ap()
    st_ap = nc.dram_tensor("stats_out", (1, 4), F32, kind="ExternalOutput").ap()
    with tile.TileContext(nc) as tc:
        _bitlinear(tc, out_ap, st_ap, xT_ap, wT_ap, wsl_ap)
    nc.compile()
    return nc


_NC_CACHE = None


def _get_nc():
    global _NC_CACHE
    if _NC_CACHE is None:
        _NC_CACHE = _build()
    return _NC_CACHE


def _run(x, weight, **spmd_kwargs):
    x = np.asarray(x, dtype=np.float32)
    w = np.asarray(weight, dtype=np.float32)
    assert x.shape == (T, I) and w.shape == (O, I)
    nc = _get_nc()
    wT = np.ascontiguousarray(w.T)  # [I, O]
    in_maps = [
        {
            # pure dtype cast + per-shard transpose; no stats on the host
            "xT_shard": np.ascontiguousarray(
                x[k * TSH : (k + 1) * TSH].T.astype(ml_dtypes.bfloat16)
            ),
            "wT_full": wT,
            "wT_slice": wT[k * ISL : (k + 1) * ISL],  # contiguous view
        }
        for k in range(NC)
    ]
    res = run_bass_kernel_spmd(nc, in_maps, core_ids=list(range(NC)), **spmd_kwargs)
    outs = res.results

    st0 = outs[0]["stats_out"][0]
    gsum, sx = float(st0[0]), float(st0[2])

    # replicate the reference's fp32 scalar arithmetic; nnz counted here by
    # replaying the device's exact fp32 threshold compare (host-consumed only)
    f32 = np.float32
    thr_dev = f32(f32(gsum) * f32(0.7 / float(O * I)))
    nnz = float(np.count_nonzero(np.abs(w) >= thr_dev))
    n_el = f32(float(O) * float(I))
    abs_mean = f32(f32(gsum) / n_el)
    non_zero_mean = f32(f32(f32(nnz) / n_el) + f32(1e-8))
    scale_w = f32(abs_mean / non_zero_mean)
    scale = f32(np.float64(scale_w) / np.float64(sx))

    # un-chunk each core's [8 panels][8 tb][128][512] bf16 output
    out = np.empty((T, O), dtype=np.float32)
    for k in range(NC):
        chunk = outs[k]["out_shard"].astype(np.float32).reshape(8, 8, P, NMM)
        out[k * TSH : (k + 1) * TSH] = (
            chunk.transpose(1, 2, 0, 3).reshape(TSH, O)
        )
    out *= scale
    return out, res


def kernel(x, weight):
    out, _ = _run(x, weight)
    return out


# revision 22
# speedup vs baseline: 1.0284x; 1.0031x over previous
"""BitLinear fake-quant GEMM on 8 TRN2 NeuronCores — fp8 DoubleRow edition.

Reference math:
  abs_mean  = mean(|W|);  thr = 0.7*abs_mean
  Wq        = sign(W) * (|W| >= thr)            (ternary)
  scale_w   = abs_mean / (mean(Wq != 0) + 1e-8)
  sx        = 127 / max(|X|)
  Xq        = round(X * sx)                      (integer valued, |.| <= 127)
  out       = (Xq @ Wq^T) * scale_w / sx

Sharding: data-parallel over tokens (8192/8 = 1024 columns of X^T per core);
W is replicated.  The host hands each core a PRE-TRANSPOSED bf16 x shard and
fp32 w^T, so both matmul operands have the contraction dim on partitions.

The GEMM runs on the PE in fp8e4 (e4m3) DoubleRow mode: one matmul
instruction contracts TWO 128-deep k-chunks at 0.5 cycles/row — 2x the bf16
FLOP rate.  Exactness: Xq (ints, |.|<=127) is split Xq = A + B with
A = fp8_rne(Xq) (e4m3-exact) and B = Xq - A (integer in [-4,4], fp8-exact);
Wq in {-1,0,1} is fp8-exact.  A@Wq + B@Wq accumulated in fp32 PSUM
reproduces Xq@Wq exactly (all products integers, sums < 2^24).  Net PE time
halves vs a bf16 kernel.

x is pre-converted to bf16 on the host (pure dtype cast, no stats): halves
x DMA and SBUF.  |x|-max is computed on device from the bf16 values; sx then
differs from the fp32-max reference by <= 2^-9 relative, worth ~3.6 max
output deviation against the 11.2 allowed by rel_err 2e-2.

Schedule (single 360 B/ns DMA device; every engine queue is in-order, and a
DMA whose semaphore wait is pending BLOCKS its queue's dispatch — used
deliberately below):
  sync-q: wsl x4, x pass-1 x16, collective-input hop (blocks until local
          stats land, keeping later transfers from queueing ahead of it at
          the FIFO DMA device), p0-q0 + two re-reads, collective-output hop
          (blocks until the AllGather lands), then p0-q1..q3 interleaved
          with re-reads, panels 1-7 quarters.  scalar-q: bf16 output writes.
  DVE:  x |max| pass (tensor_scalar + max accum_out at the 2x rate, |x| to a
        victim tile so the signed tiles survive), u = x*sx + MAGIC (2x),
        B = (u-MAGIC) - A, ternary wq for panels 1-7 (panel 1's interleaved
        into the last pairs).
  ACT:  |wsl| abs + sum accum_out, A = fp8(u - MAGIC), PSUM->bf16 copies.
  Pool: partition reduces/broadcasts, collective dispatch, b2 = (w<=-thr)
        for all panels, and panel-0's wq (keeps DVE free for the pairs the
        moment sx lands).
  PE:   panel 0 pair-major in production order (each arriving (A,B) pair
        feeds all 8 PSUM banks), panels 1-7 quarter-major.
  The last 3 pairs' x tiles are still resident from pass 1, so they need no
  re-read and quantize first, at sx+0.

Stats: per-core |W|-slice sum + |x|-shard max, one 2-scalar AllGather, local
reduce.  nnz (host-consumed only, for scale_w) is counted on the host by
replaying the device's exact fp32 threshold compare.  Output is written
bf16, tile-chunked; the host upcasts, scales by scale_w/sx and permutes.
"""

from contextlib import ExitStack

import numpy as np
import ml_dtypes

import concourse.bass as bass
import concourse.bass_isa as bass_isa
import concourse.tile as tile
from concourse import bacc, mybir
from concourse.bass import ts as _ts
from concourse.bass_utils import run_bass_kernel_spmd

P = 128
T, I, O = 8192, 4096, 4096  # tokens, in_features, out_features
NC = 8
TSH = T // NC  # 1024 token columns per core
ISL = I // NC  # 512 wT rows per core for stats
NMM = 512  # matmul moving free dim (one fp32 PSUM bank)
GF = 4096  # W staging tile free size (one [128, 4096] fp32 tile = 2 MB)
NPAIR = 16  # k-chunk pairs (32 chunks of 128 over I=4096)
NRES = 6  # trailing pairs whose pass-1 tiles stay resident (= xin bufs)
MAGIC = 12582912.0  # 1.5 * 2**23: fp32 round-to-nearest-even bias trick

F32 = mybir.dt.float32
BF16 = mybir.dt.bfloat16
FP8 = mybir.dt.float8e4
ALU = mybir.AluOpType
AXX = mybir.AxisListType
ACTF = mybir.ActivationFunctionType
DR = mybir.MatmulPerfMode.DoubleRow

# pass-1 load order: pairs 0..NRES-1 last, so they are the tiles still
# resident in the xin pool when sx lands — and they need quarter 0, which is
# also the first ternary quarter produced
PASS1_ORDER = list(range(NRES, NPAIR)) + list(range(NRES))
PAIR_ORDER = list(range(NPAIR))


def _bitlinear(tc, out, sout, xT, wT, wsl):
    nc = tc.nc
    with ExitStack() as ctx:
        const = ctx.enter_context(tc.tile_pool(name="const", bufs=1))
        statp = ctx.enter_context(tc.tile_pool(name="statp", bufs=1))
        dram = ctx.enter_context(tc.tile_pool(name="dram", bufs=1, space="DRAM"))
        stg = ctx.enter_context(tc.tile_pool(name="stg", bufs=4))    # f32 [128,4096]
        xin = ctx.enter_context(tc.tile_pool(name="xin", bufs=NRES))  # bf16 [128,2048]
        up = ctx.enter_context(tc.tile_pool(name="up", bufs=2))      # f32 [128,2048]
        abp = ctx.enter_context(tc.tile_pool(name="abp", bufs=1))    # fp8 [128,2048] x32
        wqp = ctx.enter_context(tc.tile_pool(name="wqp", bufs=5))    # fp8 [128,4096]
        b2p = ctx.enter_context(tc.tile_pool(name="b2p", bufs=2))    # fp8 [128,4096]
        psum = ctx.enter_context(tc.tile_pool(name="psum", bufs=1, space="PSUM"))
        osb = ctx.enter_context(tc.tile_pool(name="osb", bufs=4))    # bf16 [128,512]

        # Pool-engine consts first so they don't queue behind later Pool work
        nmagic128 = const.tile([P, 1], F32)
        nc.gpsimd.memset(nmagic128[:], -MAGIC)

        def xpair_src(j):
            # xT rows [2j*128, (2j+2)*128) as [128, 2 chunks, 1024 tokens]
            return xT[2 * j * P : (2 * j + 2) * P, :].rearrange(
                "(c p) t -> p c t", p=P
            )

        # ---- Phase 1: |W| slice sum (ACT Abs + accumulator) interleaved
        # with the x |max| pass (DVE tensor_reduce).  The wsl transfers are
        # spread through the x stream so the 2.2us-per-tile reduces pipeline
        # under the DMA stream instead of extending past it ----
        wsum_part = statp.tile([P, 4], F32)
        xmax_part = statp.tile([P, NPAIR], F32)
        x_tiles = [None] * NPAIR

        def emit_wsl(c):
            wt = stg.tile([P, GF], F32, tag="stg", name=f"wsl{c}")
            nc.sync.dma_start(wt[:], wsl[_ts(c, P), :])
            # in-place |w|; accum_out gives the per-partition row sum free
            nc.scalar.activation(
                wt[:], wt[:], ACTF.Abs, accum_out=wsum_part[:, c : c + 1]
            )

        def emit_x1(j):
            xt = xin.tile([P, 2 * TSH], BF16, tag="xin", name=f"x1_{j}")
            nc.sync.dma_start(
                xt[:].rearrange("p (c t) -> p c t", c=2), xpair_src(j)
            )
            nc.vector.tensor_reduce(
                xmax_part[:, j : j + 1], xt[:], axis=AXX.X, op=ALU.max,
                apply_absolute_value=True,
            )
            x_tiles[j] = xt

        for n, j in enumerate(PASS1_ORDER):
            if n in (2, 6, 10, 14):
                emit_wsl((n - 2) // 4)
            emit_x1(j)

        wsum_c = statp.tile([P, 1], F32)
        nc.vector.tensor_reduce(wsum_c[:], wsum_part[:], axis=AXX.X, op=ALU.add)
        xmax_c = statp.tile([P, 1], F32)
        nc.vector.tensor_reduce(xmax_c[:], xmax_part[:], axis=AXX.X, op=ALU.max)
        wsum_a = statp.tile([P, 1], F32)
        nc.gpsimd.partition_all_reduce(
            wsum_a[:], wsum_c[:], channels=P, reduce_op=bass_isa.ReduceOp.add
        )
        xmax_a = statp.tile([P, 1], F32)
        nc.gpsimd.partition_all_reduce(
            xmax_a[:], xmax_c[:], channels=P, reduce_op=bass_isa.ReduceOp.max
        )

        # ---- one tiny AllGather of [wsum, xmax]; reduce locally.  Both DRAM
        # hops ride the sync queue: a waiting DMA blocks the queue's
        # dispatch, so later bulk transfers cannot queue ahead of the hop at
        # the FIFO DMA device — each hop fires the moment its data is ready --
        loc = statp.tile([1, 2], F32)
        nc.vector.tensor_copy(loc[0:1, 0:1], wsum_a[0:1, 0:1])
        nc.vector.tensor_copy(loc[0:1, 1:2], xmax_a[0:1, 0:1])
        cin = dram.tile([1, 2], F32)
        cout = dram.tile([1, 2 * NC], F32)
        nc.sync.dma_start(cin[:], loc[:])
        nc.gpsimd.collective_compute(
            "AllGather", ALU.bypass, replica_groups=[list(range(NC))],
            ins=[cin.opt()], outs=[cout.opt()],
        )

        def emit_wload(op_, q, nm):
            wt = stg.tile([P, GF], F32, tag="stg", name=nm)
            src = wT[
                q * 1024 : (q + 1) * 1024, _ts(op_, NMM)
            ].rearrange("(c p) j -> p c j", p=P)
            nc.sync.dma_start(wt[:].rearrange("p (c j) -> p c j", c=8), src)
            return wt

        def emit_re(j):
            xt = xin.tile([P, 2 * TSH], BF16, tag="xin", name=f"xre{j}")
            nc.sync.dma_start(
                xt[:].rearrange("p (c t) -> p c t", c=2), xpair_src(j)
            )
            x_tiles[j] = xt

        # p0-q0 + two re-reads slot in ahead of the collective's output hop
        p0_wt = [None] * 4
        p0_wt[0] = emit_wload(0, 0, "p0w0")
        emit_re(6)
        emit_re(7)

        gg = statp.tile([1, 2 * NC], F32)
        nc.sync.dma_start(gg[:], cout[:])

        p0_wt[1] = emit_wload(0, 1, "p0w1")
        emit_re(8)
        emit_re(9)
        p0_wt[2] = emit_wload(0, 2, "p0w2")
        emit_re(10)
        emit_re(11)
        p0_wt[3] = emit_wload(0, 3, "p0w3")
        for j in range(12, NPAIR):
            emit_re(j)

        gg3 = gg[:].rearrange("a (r k) -> a r k", k=2)
        gsum = statp.tile([1, 1], F32)
        nc.vector.tensor_reduce(gsum[:], gg3[:, :, 0:1], axis=AXX.XY, op=ALU.add)
        gmax = statp.tile([1, 1], F32)
        nc.vector.tensor_reduce(gmax[:], gg3[:, :, 1:2], axis=AXX.XY, op=ALU.max)

        thr1 = statp.tile([1, 1], F32)
        nc.vector.tensor_scalar(thr1[:], gsum[:], 0.7 / float(O * I), None, op0=ALU.mult)
        nthr1 = statp.tile([1, 1], F32)
        nc.vector.tensor_scalar(nthr1[:], thr1[:], -1.0, None, op0=ALU.mult)
        thr128 = const.tile([P, 1], F32)
        nc.gpsimd.partition_broadcast(thr128[:], thr1[:])
        nthr128 = const.tile([P, 1], F32)
        nc.gpsimd.partition_broadcast(nthr128[:], nthr1[:])

        gmax_c = statp.tile([1, 1], F32)
        nc.vector.tensor_scalar(gmax_c[:], gmax[:], 1e-12, None, op0=ALU.max)
        rec1 = statp.tile([1, 1], F32)
        nc.vector.reciprocal(rec1[:], gmax_c[:])
        sx1 = statp.tile([1, 1], F32)
        nc.vector.tensor_scalar(sx1[:], rec1[:], 127.0, None, op0=ALU.mult)
        sx128 = const.tile([P, 1], F32)
        nc.gpsimd.partition_broadcast(sx128[:], sx1[:])

        def emit_wq_dve(wt, b2, nm):
            wq = wqp.tile([P, GF], FP8, tag="wq", name=nm)
            nc.vector.scalar_tensor_tensor(
                wq[:], wt[:], thr128[:], b2[:], op0=ALU.is_ge, op1=ALU.subtract
            )
            return wq

        # ---- panel-0 quant: quarters 0+1 ternary on DVE ahead of the pair
        # stream (q0's b2 too — no Pool dependency on the critical path);
        # quarters 2+3 fully on Pool, landing well before pairs 8 and 12 ----
        p0_wq = [None] * 4
        b2q0 = b2p.tile([P, GF], FP8, tag="b2", name="p0b2_0")
        nc.vector.tensor_scalar(
            b2q0[:], p0_wt[0][:], nthr128[:], None, op0=ALU.is_le
        )
        p0_wq[0] = emit_wq_dve(p0_wt[0], b2q0, "p0wq0")
        p0_b2 = [b2q0]
        for q in range(1, 4):
            b2 = b2p.tile([P, GF], FP8, tag="b2", name=f"p0b2{q}")
            nc.gpsimd.tensor_scalar(
                b2[:], p0_wt[q][:], nthr128[:], None, op0=ALU.is_le
            )
            p0_b2.append(b2)


        # ---- Phase 2: Xq = A + B split, per k-chunk pair, resident first ----
        a_tiles = [None] * NPAIR
        b_tiles = [None] * NPAIR

        def emit_quant(j, pool=False):
            ueng = nc.gpsimd if pool else nc.vector
            eng = nc.vector
            xt = x_tiles[j]
            ut = up.tile([P, 2 * TSH], F32, tag="u", name=f"u{j}")
            # u = x*sx + MAGIC: forces RNE to integer in the low mantissa
            # (tensor_scalar earns the DVE 2x rate; stt does not)
            ueng.tensor_scalar(
                ut[:], xt[:], sx128[:], MAGIC, op0=ALU.mult, op1=ALU.add
            )
            ag = abp.tile([P, 2 * TSH], FP8, tag=f"a{j}", name=f"a{j}")
            # A = fp8_rne(u - MAGIC): ACT affine is exact fp32; the fp8
            # convert rounds the integer Xq to the e4m3 grid
            nc.scalar.activation(ag[:], ut[:], ACTF.Identity, bias=nmagic128[:])
            bg = abp.tile([P, 2 * TSH], FP8, tag=f"b{j}", name=f"b{j}")
            # B = (u - MAGIC) - A: integer in [-4, 4], exactly fp8
            eng.scalar_tensor_tensor(
                bg[:], ut[:], -MAGIC, ag[:], op0=ALU.add, op1=ALU.subtract
            )
            a_tiles[j], b_tiles[j] = ag, bg

        # q2/q3 ternaries (DVE stt — not a valid Pool opcode) interleave
        # into the pair stream just ahead of the matmuls that need them
        for j in PAIR_ORDER[:2]:
            emit_quant(j, pool=(j % 2 == 1))
        p0_wq[1] = emit_wq_dve(p0_wt[1], p0_b2[1], "p0wq1")
        for j in PAIR_ORDER[2:6]:
            emit_quant(j, pool=(j % 2 == 1))
        p0_wq[2] = emit_wq_dve(p0_wt[2], p0_b2[2], "p0wq2")
        for j in PAIR_ORDER[6:10]:
            emit_quant(j, pool=(j % 2 == 1))
        p0_wq[3] = emit_wq_dve(p0_wt[3], p0_b2[3], "p0wq3")
        for j in PAIR_ORDER[10:]:
            emit_quant(j, pool=(j % 2 == 1))

        # panel-1 quarters: loads behind the re-reads on the sync queue, b2
        # on Pool behind its pair work, wq on DVE right after the pairs
        p1_wt = [emit_wload(1, q, f"p1w{q}") for q in range(4)]
        p1_wq = []
        for q in range(4):
            b2 = b2p.tile([P, GF], FP8, tag="b2", name=f"p1b2{q}")
            nc.gpsimd.tensor_scalar(
                b2[:], p1_wt[q][:], nthr128[:], None, op0=ALU.is_le
            )
            p1_wq.append(emit_wq_dve(p1_wt[q], b2, f"p1wq{q}"))

        def lhsT(tiles, j, tb):
            return tiles[j][:].rearrange("p (c t) -> p c t", c=2)[
                :, :, tb * P : (tb + 1) * P
            ]

        def wq_pair(wq, jj):
            return wq[:].rearrange("p (c j) -> p c j", c=8)[:, 2 * jj : 2 * jj + 2, :]

        # ---- panel 0, pair-major in production order ----
        ps_tiles = [
            psum.tile([P, NMM], F32, tag=f"ps{tb}", name=f"p0ps{tb}")
            for tb in range(8)
        ]
        for n, j in enumerate(PAIR_ORDER):
            q, jj = j // 4, j % 4
            for tb in range(8):
                nc.tensor.matmul(
                    ps_tiles[tb][:], lhsT=lhsT(a_tiles, j, tb),
                    rhs=wq_pair(p0_wq[q], jj),
                    start=(n == 0), stop=False, perf_mode=DR,
                )
                nc.tensor.matmul(
                    ps_tiles[tb][:], lhsT=lhsT(b_tiles, j, tb),
                    rhs=wq_pair(p0_wq[q], jj),
                    start=False, stop=(n == NPAIR - 1), perf_mode=DR,
                )
        for tb in range(8):
            ot = osb.tile([P, NMM], BF16, tag="osb")
            nc.scalar.copy(ot[:], ps_tiles[tb][:])
            nc.scalar.dma_start(out[_ts(tb, P), :], ot[:])

        # ---- panels 1-7, quarter-major; b2 on Pool, wq on DVE ----
        for op_ in range(1, 8):
            if op_ == 1:
                quarters = p1_wq
            else:
                quarters = []
                for q in range(4):
                    wt = emit_wload(op_, q, f"p{op_}w{q}")
                    b2 = b2p.tile([P, GF], FP8, tag="b2")
                    nc.gpsimd.tensor_scalar(
                        b2[:], wt[:], nthr128[:], None, op0=ALU.is_le
                    )
                    quarters.append(emit_wq_dve(wt, b2, f"p{op_}wq{q}"))
            ps_tiles = [
                psum.tile([P, NMM], F32, tag=f"ps{tb}", name=f"p{op_}ps{tb}")
                for tb in range(8)
            ]
            # last panel runs token-block-major so each PSUM bank finishes
            # (and drains) as early as possible — all quarters are ready by
            # then.  Earlier panels stay quarter-major to chase the wq stream.
            if op_ == 7:
                loops = [(tb, q, jj) for tb in range(8) for q in range(4) for jj in range(4)]
            else:
                loops = [(tb, q, jj) for q in range(4) for tb in range(8) for jj in range(4)]
            for tb, q, jj in loops:
                j = q * 4 + jj
                nc.tensor.matmul(
                    ps_tiles[tb][:], lhsT=lhsT(a_tiles, j, tb),
                    rhs=wq_pair(quarters[q], jj),
                    start=(q == 0 and jj == 0), stop=False, perf_mode=DR,
                )
                nc.tensor.matmul(
                    ps_tiles[tb][:], lhsT=lhsT(b_tiles, j, tb),
                    rhs=wq_pair(quarters[q], jj),
                    start=False, stop=(q == 3 and jj == 3), perf_mode=DR,
                )
            for tb in range(8):
                ot = osb.tile([P, NMM], BF16, tag="osb")
                nc.scalar.copy(ot[:], ps_tiles[tb][:])
                nc.scalar.dma_start(out[_ts(op_ * 8 + tb, P), :], ot[:])

        # stats out for the host (nothing on-device consumes these)
        nc.gpsimd.dma_start(sout[0:1, 0:1], gsum[:])
        nc.gpsimd.dma_start(sout[0:1, 1:2], gmax[:])
        nc.gpsimd.dma_start(sout[0:1, 2:3], sx1[:])


def _build():
    nc = bacc.Bacc("TRN2", debug=False, enable_asserts=False, num_devices=NC)
    xT_ap = nc.dram_tensor("xT_shard", (I, TSH), BF16, kind="ExternalInput").ap()
    wT_ap = nc.dram_tensor("wT_full", (I, O), F32, kind="ExternalInput").ap()
    wsl_ap = nc.dram_tensor("wT_slice", (ISL, O), F32, kind="ExternalInput").ap()
    # chunked layout: row (panel*8 + tb)*128 + r, col c  <->  out[tb*128+r, panel*512+c]
    out_ap = nc.dram_tensor("out_shard", (64 * P, NMM), BF16, kind="ExternalOutput").ap()
    st_ap = nc.dram_tensor("stats_out", (1, 4), F32, kind="ExternalOutput").ap()
    with tile.TileContext(nc) as tc:
        _bitlinear(tc, out_ap, st_ap, xT_ap, wT_ap, wsl_ap)
    nc.compile()
    return nc


_NC_CACHE = None


def _get_nc():
    global _NC_CACHE
    if _NC_CACHE is None:
        _NC_CACHE = _build()
    return _NC_CACHE


def _run(x, weight, **spmd_kwargs):
    x = np.asarray(x, dtype=np.float32)
    w = np.asarray(weight, dtype=np.float32)
    assert x.shape == (T, I) and w.shape == (O, I)
    nc = _get_nc()
    wT = np.ascontiguousarray(w.T)  # [I, O]
    in_maps = [
        {
            # pure dtype cast + per-shard transpose; no stats on the host
            "xT_shard": np.ascontiguousarray(
                x[k * TSH : (k + 1) * TSH].T.astype(ml_dtypes.bfloat16)
            ),
            "wT_full": wT,
            "wT_slice": wT[k * ISL : (k + 1) * ISL],  # contiguous view
        }
        for k in range(NC)
    ]
    res = run_bass_kernel_spmd(nc, in_maps, core_ids=list(range(NC)), **spmd_kwargs)
    outs = res.results

    st0 = outs[0]["stats_out"][0]
    gsum, sx = float(st0[0]), float(st0[2])

    # replicate the reference's fp32 scalar arithmetic; nnz counted here by
    # replaying the device's exact fp32 threshold compare (host-consumed only)
    f32 = np.float32
    thr_dev = f32(f32(gsum) * f32(0.7 / float(O * I)))
    nnz = float(np.count_nonzero(np.abs(w) >= thr_dev))
    n_el = f32(float(O) * float(I))
    abs_mean = f32(f32(gsum) / n_el)
    non_zero_mean = f32(f32(f32(nnz) / n_el) + f32(1e-8))
    scale_w = f32(abs_mean / non_zero_mean)
    scale = f32(np.float64(scale_w) / np.float64(sx))

    # un-chunk each core's [8 panels][8 tb][128][512] bf16 output
    out = np.empty((T, O), dtype=np.float32)
    for k in range(NC):
        chunk = outs[k]["out_shard"].astype(np.float32).reshape(8, 8, P, NMM)
        out[k * TSH : (k + 1) * TSH] = (
            chunk.transpose(1, 2, 0, 3).reshape(TSH, O)
        )
    out *= scale
    return out, res


def kernel(x, weight):
    out, _ = _run(x, weight)
    return out
